# revision 39
# baseline (speedup 1.0000x reference)
"""CQAttention (QANet context-query attention) Bass kernel for 8 Trainium2 cores.

Math (per batch, masks all-ones, eval mode):
  Ct = C.T [Lc,D], Qt = Q.T [Lq,D]
  S  = Ct@w4C + (Qt@w4Q).T + (Ct*w4mlu)@Qt.T + bias          [Lc,Lq]
  S1 = softmax_q(S), S2 = softmax_c(S)
  A  = S1@Qt ; Bt = S1@(S2.T@Ct)
  out = concat([Ct, A, Ct*A, Ct*Bt], -1).T                    [4D, Lc]

Key reductions:
  - (S1@S2.T)@Ct re-associated as S1@(S2.T@Ct)  (6x fewer flops)
  - softmax terms constant along the reduced axis cancel:
      S1 = E1/r,  E1^T[q,c] = exp(sum_d Qm[d,q]*C[d,c] + b[q]),
                  Qm = Q*w4mlu,  b = Q.T@w4Q  (applied as Act-engine bias)
      S2 = E2/s,  E2[c,q]   = exp(sum_d C[d,c]*Qaug[d,q]),  Qaug = Q*w4mlu + w4C
  - row-sums r replicated across partitions for free via ones-matmul
  - outputs stay in [d, c] layout end-to-end:
      out1 = MA*(1/r), out2 = MA*(C/r), out3 = MB*(C/r)
      MA = Qt.T @ E1^T, MB = T.T @ E1^T, T = (Ct.T @ E2).T * (1/s)

Perf structure (single-run critical path oriented):
  - C/Q used as matmul operands via .bitcast(float32r) — no rounding copies.
  - transposes stream a bf16 identity (1 cy/row instead of 2).
  - plane-0 output (C passthrough) is a DRAM->DRAM DMA, zero SBUF deps.
  - loads (SP queue) prefetch all 4 batches up-front (Csb/Qsb bufs=4);
    half-stores issue from the DVE queue right after their last mul, so a
    blocked store can never head-of-line-block the loads.
  - work tiles double-buffered so consecutive batches pipeline.
"""

import numpy as np

import concourse.bass as bass
import concourse.bacc as bacc
import concourse.tile as tile
from concourse import mybir
from contextlib import ExitStack, nullcontext

B, D, LC, LQ = 32, 128, 2048, 256
NCORES = 8
BPC = B // NCORES  # batches per core

F32 = mybir.dt.float32
F32R = mybir.dt.float32r
BF16 = mybir.dt.bfloat16
AF = mybir.ActivationFunctionType
ALU = mybir.AluOpType

CQ_BUFS = 4      # Csb/Qsb input rings (deep prefetch: all 4 batches)
OUT_BUFS = 9     # per-plane output staging rings ([128,1024] each)
WORK_BUFS = 2    # intermediate rings (pipeline adjacent batches)
FRONT_BUFS = 2   # 1024-col psum ring: E2/E1/Ct (2 banks each)
BACK_BUFS = 2    # 512-col psum ring: bvec/qt/r/MA/MB (1 bank each)
SMALL_BUFS = 2   # 512-col psum ring: s/MT/T chain (1 bank each)


def build_nc(reps=1, hw_loop=False):
    nc = bacc.Bacc("TRN2", target_bir_lowering=False)
    C_in = nc.declare_dram_parameter("C", [BPC, D, LC], F32, isOutput=False)
    Q_in = nc.declare_dram_parameter("Q", [BPC, D, LQ], F32, isOutput=False)
    w4C_in = nc.declare_dram_parameter("w4C", [D, 1], F32, isOutput=False)
    w4Q_in = nc.declare_dram_parameter("w4Q", [D, 1], F32, isOutput=False)
    w4mlu_in = nc.declare_dram_parameter("w4mlu", [D, 1], F32, isOutput=False)
    out_ext = nc.declare_dram_parameter("out", [BPC, 4 * D, LC], F32, isOutput=True)

    with ExitStack() as ctx:
        tc = ctx.enter_context(tile.TileContext(nc))
        singles = ctx.enter_context(tc.tile_pool(name="singles", bufs=1))
        io = ctx.enter_context(tc.tile_pool(name="io", bufs=1))
        work = ctx.enter_context(tc.tile_pool(name="work", bufs=1))
        psum = ctx.enter_context(tc.tile_pool(name="psum", bufs=1, space="PSUM"))

        # --- singles: identity (bf16 for 1cy/row transposes), ones, weights ---
        ident = singles.tile([128, 128], F32)
        nc.gpsimd.memset(ident, 0.0)
        nc.gpsimd.affine_select(
            out=ident, in_=ident, compare_op=ALU.not_equal, fill=1.0,
            base=0, pattern=[[-1, 128]], channel_multiplier=1)
        ones_f = singles.tile([128, 128], F32)
        nc.vector.memset(ones_f, 1.0)
        ones = singles.tile([128, 128], F32R)
        nc.vector.tensor_copy(out=ones, in_=ones_f)
        w4mlu_sb = singles.tile([128, 1], F32)
        w4C_sb = singles.tile([128, 1], F32)
        w4Q_sb = singles.tile([128, 1], F32)
        # Act engine exp-table preload, off the critical path
        warm = singles.tile([128, 1], F32)
        nc.vector.memset(warm, 0.0)
        nc.scalar.activation(out=warm, in_=warm, func=AF.Exp)

        loop_cm = (tc.For_i(0, reps, 1,
                            hint_engines=(mybir.EngineType.PE,
                                          mybir.EngineType.DVE,
                                          mybir.EngineType.Activation,
                                          mybir.EngineType.SP,
                                          mybir.EngineType.Pool))
                   if hw_loop else nullcontext(0))
        with loop_cm:
         for rep in range(1 if hw_loop else reps):
          # ---- all loads up-front (SP queue; bufs=BPC -> no WAR waits),
          # passthroughs interleaved behind so loads win the DMA engine ----
          Csbs, Qsbs = [], []
          for b in range(BPC):
              if b == 0 and rep == 0:
                  # Qaug's weights first so E2 is gated only by the C load
                  nc.sync.dma_start(out=w4mlu_sb, in_=w4mlu_in[:])
                  nc.sync.dma_start(out=w4C_sb, in_=w4C_in[:])
              Qsb = io.tile([128, LQ], F32R, tag="Qsb", bufs=CQ_BUFS)
              nc.sync.dma_start(out=Qsb, in_=Q_in[b].bitcast(F32R))
              Csb = io.tile([128, LC], F32R, tag="Csb", bufs=CQ_BUFS)
              nc.sync.dma_start(out=Csb, in_=C_in[b].bitcast(F32R))
              if b == 0 and rep == 0:
                  nc.sync.dma_start(out=w4Q_sb, in_=w4Q_in[:])
              Csbs.append(Csb)
              Qsbs.append(Qsb)
              if b > 0:
                  # plane-0 passthrough: DRAM->DRAM, no SBUF dependency
                  nc.sync.dma_start(out=out_ext[b - 1, 0:128, :],
                                    in_=C_in[b - 1])
          nc.sync.dma_start(out=out_ext[BPC - 1, 0:128, :],
                            in_=C_in[BPC - 1])

          for b in range(BPC):
              Csb = Csbs[b]
              Qsb = Qsbs[b]
              Cr = Csb
              Qr = Qsb
              Cf = Csb.bitcast(F32)
              Qf = Qsb.bitcast(F32)

              # ---- small Q-side prep (DVE) ----
              Qm = work.tile([128, LQ], F32R, tag="Qm", bufs=WORK_BUFS)
              nc.gpsimd.tensor_scalar(
                  out=Qm, in0=Qf, scalar1=w4mlu_sb, scalar2=None, op0=ALU.mult)
              Qaug = work.tile([128, LQ], F32R, tag="Qaug", bufs=WORK_BUFS)
              nc.gpsimd.tensor_scalar(
                  out=Qaug, in0=Qf, scalar1=w4mlu_sb, scalar2=w4C_sb,
                  op0=ALU.mult, op1=ALU.add)

              # ---- E2[c,q] = exp(C.T @ Qaug): c-tile j at cols 256j ----
              # first PE work of the batch: its front-ring slots were freed
              # by batch b-1's E1 psums (early), never by late-batch work
              E2 = work.tile([128, 16 * LQ], F32R, tag="E2", bufs=WORK_BUFS)
              for g in range(4):
                  ps = psum.tile([128, 1024], F32, tag="front", bufs=FRONT_BUFS)
                  for j in range(4):
                      ctile = g * 4 + j
                      nc.tensor.matmul(
                          ps[:, 256 * j:256 * (j + 1)],
                          Cr[:, 128 * ctile:128 * (ctile + 1)], Qaug,
                          start=True, stop=True)
                  nc.scalar.activation(
                      out=E2[:, 1024 * g:1024 * (g + 1)], in_=ps, func=AF.Exp)

              # ---- b[q] = Q.T @ w4Q (2 tiny matmuls), Act copy to SBUF ----
              ps_b = psum.tile([128, 512], F32, tag="back", bufs=BACK_BUFS)
              for qt in range(2):
                  nc.tensor.matmul(
                      ps_b[:, qt:qt + 1], Qf[:, 128 * qt:128 * (qt + 1)],
                      w4Q_sb, start=True, stop=True)
              b_sb = work.tile([128, 2], F32, tag="b_sb", bufs=WORK_BUFS)
              nc.scalar.copy(out=b_sb, in_=ps_b[:, 0:2])

              # ---- Qt = Q.T (2 transposes), Pool copy ----
              ps_qt = psum.tile([128, 512], F32, tag="back", bufs=BACK_BUFS)
              for j in range(2):
                  nc.tensor.transpose(
                      ps_qt[:, 128 * j:128 * (j + 1)],
                      Qf[:, 128 * j:128 * (j + 1)], ident)
              Qt = work.tile([128, LQ], F32R, tag="Qt", bufs=WORK_BUFS)
              nc.scalar.copy(out=Qt, in_=ps_qt[:, 0:256])

              # ---- E1^T[q,c] = exp(Qm.T @ C + b[q]): q-tile qt at 2048*qt ----
              E1 = work.tile([128, 2 * LC], F32R, tag="E1", bufs=WORK_BUFS)
              for qt in range(2):
                  for g in range(2):
                      ps = psum.tile([128, 1024], F32, tag="front",
                                     bufs=FRONT_BUFS)
                      for cc in range(2):
                          c0 = 1024 * g + 512 * cc
                          nc.tensor.matmul(
                              ps[:, 512 * cc:512 * (cc + 1)],
                              Qm[:, 128 * qt:128 * (qt + 1)],
                              Cr[:, c0:c0 + 512],
                              start=True, stop=True)
                      nc.scalar.activation(
                          out=E1[:, 2048 * qt + 1024 * g:2048 * qt + 1024 * (g + 1)],
                          in_=ps, func=AF.Exp, bias=b_sb[:, qt:qt + 1])

              # ---- Ct = C.T (16 transposes), Pool copies ----
              Ct = work.tile([128, LC], F32R, tag="Ct", bufs=WORK_BUFS)
              for g in range(2):
                  ps_ct = psum.tile([128, 1024], F32, tag="front",
                                    bufs=FRONT_BUFS)
                  for j in range(8):
                      cj = g * 8 + j
                      nc.tensor.transpose(
                          ps_ct[:, 128 * j:128 * (j + 1)],
                          Cf[:, 128 * cj:128 * (cj + 1)], ident)
                  nc.scalar.copy(
                      out=Ct[:, 1024 * g:1024 * (g + 1)], in_=ps_ct)

              # ---- s (col-sums of E2, replicated) -> rs = 1/s row ----
              # s sits replicated in every partition, so 1/s works as a
              # free-axis row vector; no transposes, no DVE in the T-chain
              # beyond this early reciprocal
              ps_s = psum.tile([128, 512], F32, tag="small", bufs=SMALL_BUFS)
              for j in range(16):
                  nc.tensor.matmul(
                      ps_s[:, 0:256], ones, E2[:, 256 * j:256 * (j + 1)],
                      start=(j == 0), stop=(j == 15))
              rs = work.tile([128, LQ], F32, tag="rs", bufs=WORK_BUFS)
              nc.vector.reciprocal(out=rs, in_=ps_s[:, 0:256])

              # ---- MT^T = Ct.T @ E2; T^T = MT^T * (1/s) fused on Pool ----
              MTts = work.tile([128, LQ], F32R, tag="MTts", bufs=WORK_BUFS)
              ps_mt = psum.tile([128, 512], F32, tag="small", bufs=SMALL_BUFS)
              for j in range(16):
                  nc.tensor.matmul(
                      ps_mt[:, 0:256],
                      Ct[:, 128 * j:128 * (j + 1)], E2[:, 256 * j:256 * (j + 1)],
                      start=(j == 0), stop=(j == 15))
              nc.vector.scalar_tensor_tensor(
                  out=MTts, in0=ps_mt[:, 0:256], scalar=1.0, in1=rs,
                  op0=ALU.mult, op1=ALU.mult)

              # ---- T_sb[q,d] = transpose(T^T), Act copy from psum ----
              T_sb = work.tile([128, LQ], F32R, tag="T_sb", bufs=WORK_BUFS)
              ps_t = psum.tile([128, 512], F32, tag="small", bufs=SMALL_BUFS)
              for j in range(2):
                  nc.tensor.transpose(
                      ps_t[:, 128 * j:128 * (j + 1)],
                      MTts[:, 128 * j:128 * (j + 1)].bitcast(F32), ident)
              nc.scalar.copy(out=T_sb, in_=ps_t[:, 0:256])

              # ---- r (row-sums of E1 over q, replicated) -> rbi = 1/r ----
              rbi = work.tile([128, LC], F32, tag="rbi", bufs=WORK_BUFS)
              for gc in range(4):
                  c0 = 512 * gc
                  ps = psum.tile([128, 512], F32, tag="back", bufs=BACK_BUFS)
                  for qt in range(2):
                      nc.tensor.matmul(
                          ps, ones, E1[:, 2048 * qt + c0:2048 * qt + c0 + 512],
                          start=(qt == 0), stop=(qt == 1))
                  nc.vector.reciprocal_approx_fast(
                      out=rbi[:, c0:c0 + 512], in_=ps)

              # Crbi = C * (1/r) on Pool, split per half for earlier starts
              Crbi = work.tile([128, LC], F32, tag="Crbi", bufs=WORK_BUFS)
              for g in range(2):
                  sl = slice(1024 * g, 1024 * (g + 1))
                  nc.gpsimd.tensor_mul(out=Crbi[:, sl], in0=Cf[:, sl],
                                       in1=rbi[:, sl])

              # ---- per column-half: MA -> out1/out2, MB -> out3 ----
              # per-plane staging tiles + per-plane stores: each plane ships
              # as soon as its two muls finish (keeps DMA fed, short tail)
              for g in range(2):
                with G(6 + g):
                  sl = slice(1024 * g, 1024 * (g + 1))
                  P1 = io.tile([128, 1024], F32, tag="outp", bufs=OUT_BUFS)
                  P2 = io.tile([128, 1024], F32, tag="outp", bufs=OUT_BUFS)
                  P3 = io.tile([128, 1024], F32, tag="outp", bufs=OUT_BUFS)
                  for cc in range(2):
                      c0 = 1024 * g + 512 * cc
                      scc = slice(512 * cc, 512 * (cc + 1))
                      psA = psum.tile([128, 512], F32, tag="back",
                                      bufs=BACK_BUFS)
                      for qt in range(2):
                          nc.tensor.matmul(
                              psA,
                              Qt[:, 128 * qt:128 * (qt + 1)],
                              E1[:, 2048 * qt + c0:2048 * qt + c0 + 512],
                              start=(qt == 0), stop=(qt == 1))
                      nc.vector.tensor_mul(out=P1[:, scc], in0=psA,
                                           in1=rbi[:, c0:c0 + 512])
                      nc.vector.tensor_mul(out=P2[:, scc], in0=psA,
                                           in1=Crbi[:, c0:c0 + 512])
                      nc.sync.dma_start(out=out_ext[b, 128:256, c0:c0 + 512],
                                        in_=P1[:, scc])
                      nc.sync.dma_start(out=out_ext[b, 256:384, c0:c0 + 512],
                                        in_=P2[:, scc])
                  for cc in range(2):
                      c0 = 1024 * g + 512 * cc
                      scc = slice(512 * cc, 512 * (cc + 1))
                      psB = psum.tile([128, 512], F32, tag="back",
                                      bufs=BACK_BUFS)
                      for qt in range(2):
                          nc.tensor.matmul(
                              psB,
                              T_sb[:, 128 * qt:128 * (qt + 1)],
                              E1[:, 2048 * qt + c0:2048 * qt + c0 + 512],
                              start=(qt == 0), stop=(qt == 1))
                      nc.vector.tensor_mul(out=P3[:, scc], in0=psB,
                                           in1=Crbi[:, c0:c0 + 512])
                      nc.sync.dma_start(out=out_ext[b, 384:512, c0:c0 + 512],
                                        in_=P3[:, scc])

    nc.compile()
    return nc


_NC = {}


def _get_nc(reps=1, hw_loop=False):
    key = (reps, hw_loop)
    if key not in _NC:
        _NC[key] = build_nc(reps, hw_loop)
    return _NC[key]


def make_in_maps(C, Q, w4C, w4Q, w4mlu):
    C = np.ascontiguousarray(np.asarray(C), dtype=np.float32)
    Q = np.ascontiguousarray(np.asarray(Q), dtype=np.float32)
    w4C = np.ascontiguousarray(np.asarray(w4C), dtype=np.float32).reshape(D, 1)
    w4Q = np.ascontiguousarray(np.asarray(w4Q), dtype=np.float32).reshape(D, 1)
    w4mlu = np.ascontiguousarray(np.asarray(w4mlu), dtype=np.float32).reshape(D, 1)
    in_maps = []
    for i in range(NCORES):
        sl = slice(i * BPC, (i + 1) * BPC)
        in_maps.append({
            "C": np.ascontiguousarray(C[sl]),
            "Q": np.ascontiguousarray(Q[sl]),
            "w4C": w4C, "w4Q": w4Q, "w4mlu": w4mlu,
        })
    return in_maps


def run(C, Q, w4C, w4Q, w4mlu, trace=False, tmpdir=None):
    from concourse.bass_utils import run_bass_kernel_spmd
    nc = _get_nc()
    in_maps = make_in_maps(C, Q, w4C, w4Q, w4mlu)
    res = run_bass_kernel_spmd(
        nc, in_maps, list(range(NCORES)), trace=trace, tmpdir=tmpdir)
    out = np.concatenate(
        [res.results[i]["out"] for i in range(NCORES)], axis=0)
    return out, res


def kernel(C, Q, Cmask=None, Qmask=None, w4C=None, w4Q=None, w4mlu=None,
           bias=None, **_unused):
    # Cmask/Qmask are all-ones in this problem and bias cancels exactly in
    # every output (softmax shift invariance), so neither reaches the device.
    out, _ = run(C, Q, w4C, w4Q, w4mlu)
    return out


# revision 48
# speedup vs baseline: 122.2196x; 122.2196x over previous
"""CQAttention (QANet context-query attention) Bass kernel for 8 Trainium2 cores.

Math (per batch, masks all-ones, eval mode):
  Ct = C.T [Lc,D], Qt = Q.T [Lq,D]
  S  = Ct@w4C + (Qt@w4Q).T + (Ct*w4mlu)@Qt.T + bias          [Lc,Lq]
  S1 = softmax_q(S), S2 = softmax_c(S)
  A  = S1@Qt ; Bt = S1@(S2.T@Ct)
  out = concat([Ct, A, Ct*A, Ct*Bt], -1).T                    [4D, Lc]

Key reductions:
  - (S1@S2.T)@Ct re-associated as S1@(S2.T@Ct)  (6x fewer flops)
  - softmax terms constant along the reduced axis cancel:
      S1 = E1/r,  E1^T[q,c] = exp(sum_d Qm[d,q]*C[d,c] + b[q]),
                  Qm = Q*w4mlu,  b = Q.T@w4Q  (applied as Act-engine bias)
      S2 = E2/s,  E2[c,q]   = exp(sum_d C[d,c]*Qaug[d,q]),  Qaug = Q*w4mlu + w4C
  - row-sums r replicated across partitions for free via ones-matmul
  - outputs stay in [d, c] layout end-to-end:
      out1 = MA*(1/r), out2 = MA*(C/r), out3 = MB*(C/r)
      MA = Qt.T @ E1^T, MB = T.T @ E1^T, T = (Ct.T @ E2).T * (1/s)

Perf structure (single-run critical path oriented):
  - C/Q used as matmul operands via .bitcast(float32r) — no rounding copies.
  - transposes stream a bf16 identity (1 cy/row instead of 2).
  - plane-0 output (C passthrough) is a DRAM->DRAM DMA, zero SBUF deps.
  - loads (SP queue) prefetch all 4 batches up-front (Csb/Qsb bufs=4);
    half-stores issue from the DVE queue right after their last mul, so a
    blocked store can never head-of-line-block the loads.
  - work tiles double-buffered so consecutive batches pipeline.
"""

import numpy as np

import concourse.bass as bass
import concourse.bacc as bacc
import concourse.tile as tile
from concourse import mybir
from contextlib import ExitStack, nullcontext

B, D, LC, LQ = 32, 128, 2048, 256
NCORES = 8
BPC = B // NCORES  # batches per core

F32 = mybir.dt.float32
F32R = mybir.dt.float32r
BF16 = mybir.dt.bfloat16
AF = mybir.ActivationFunctionType
ALU = mybir.AluOpType

CQ_BUFS = 4      # Csb/Qsb input rings (deep prefetch: all 4 batches)
OUT_BUFS = 9     # per-plane output staging rings ([128,1024] each)
WORK_BUFS = 2    # intermediate rings (pipeline adjacent batches)
FRONT_BUFS = 2   # 1024-col psum ring: E2/E1/Ct (2 banks each)
BACK_BUFS = 2    # 512-col psum ring: bvec/qt/r/MA/MB (1 bank each)
SMALL_BUFS = 2   # 512-col psum ring: s/MT/T chain (1 bank each)


def build_nc(reps=1, hw_loop=False):
    nc = bacc.Bacc("TRN2", target_bir_lowering=False)
    C_in = nc.declare_dram_parameter("C", [BPC, D, LC], F32, isOutput=False)
    Q_in = nc.declare_dram_parameter("Q", [BPC, D, LQ], F32, isOutput=False)
    w4C_in = nc.declare_dram_parameter("w4C", [D, 1], F32, isOutput=False)
    w4Q_in = nc.declare_dram_parameter("w4Q", [D, 1], F32, isOutput=False)
    w4mlu_in = nc.declare_dram_parameter("w4mlu", [D, 1], F32, isOutput=False)
    out_ext = nc.declare_dram_parameter("out", [BPC, 4 * D, LC], F32, isOutput=True)

    with ExitStack() as ctx:
        tc = ctx.enter_context(tile.TileContext(nc))
        singles = ctx.enter_context(tc.tile_pool(name="singles", bufs=1))
        io = ctx.enter_context(tc.tile_pool(name="io", bufs=1))
        work = ctx.enter_context(tc.tile_pool(name="work", bufs=1))
        psum = ctx.enter_context(tc.tile_pool(name="psum", bufs=1, space="PSUM"))

        # --- singles: identity (bf16 for 1cy/row transposes), ones, weights ---
        ident = singles.tile([128, 128], F32)
        nc.gpsimd.memset(ident, 0.0)
        nc.gpsimd.affine_select(
            out=ident, in_=ident, compare_op=ALU.not_equal, fill=1.0,
            base=0, pattern=[[-1, 128]], channel_multiplier=1)
        ones_f = singles.tile([128, 128], F32)
        nc.vector.memset(ones_f, 1.0)
        ones = singles.tile([128, 128], F32R)
        nc.vector.tensor_copy(out=ones, in_=ones_f)
        w4mlu_sb = singles.tile([128, 1], F32)
        w4C_sb = singles.tile([128, 1], F32)
        w4Q_sb = singles.tile([128, 1], F32)
        # Act engine exp-table preload, off the critical path
        warm = singles.tile([128, 1], F32)
        nc.vector.memset(warm, 0.0)
        nc.scalar.activation(out=warm, in_=warm, func=AF.Exp)
        nc.sync.dma_start(out=w4mlu_sb, in_=w4mlu_in[:])
        nc.sync.dma_start(out=w4C_sb, in_=w4C_in[:])
        nc.sync.dma_start(out=w4Q_sb, in_=w4Q_in[:])

        loop_cm = (tc.For_i(0, reps, 1,
                            hint_engines=(mybir.EngineType.PE,
                                          mybir.EngineType.DVE,
                                          mybir.EngineType.Activation,
                                          mybir.EngineType.SP,
                                          mybir.EngineType.Pool))
                   if hw_loop else nullcontext(0))
        with loop_cm:
         for rep in range(1 if hw_loop else reps):
          # ---- all loads up-front (SP queue; bufs=BPC -> no WAR waits),
          # passthroughs interleaved behind so loads win the DMA engine ----
          Csbs, Qsbs = [], []
          for b in range(BPC):
              if b == 0 and rep == 0:
                  # Qaug's weights first so E2 is gated only by the C load
                  nc.sync.dma_start(out=w4mlu_sb, in_=w4mlu_in[:])
                  nc.sync.dma_start(out=w4C_sb, in_=w4C_in[:])
              Qsb = io.tile([128, LQ], F32R, tag="Qsb", bufs=CQ_BUFS)
              nc.sync.dma_start(out=Qsb, in_=Q_in[b].bitcast(F32R))
              Csb = io.tile([128, LC], F32R, tag="Csb", bufs=CQ_BUFS)
              nc.sync.dma_start(out=Csb, in_=C_in[b].bitcast(F32R))
              if b == 0 and rep == 0:
                  nc.sync.dma_start(out=w4Q_sb, in_=w4Q_in[:])
              Csbs.append(Csb)
              Qsbs.append(Qsb)
              if b > 0:
                  # plane-0 passthrough: DRAM->DRAM, no SBUF dependency
                  nc.sync.dma_start(out=out_ext[b - 1, 0:128, :],
                                    in_=C_in[b - 1])
          nc.sync.dma_start(out=out_ext[BPC - 1, 0:128, :],
                            in_=C_in[BPC - 1])

          for b in range(BPC):
              Csb = Csbs[b]
              Qsb = Qsbs[b]
              Cr = Csb
              Qr = Qsb
              Cf = Csb.bitcast(F32)
              Qf = Qsb.bitcast(F32)

              # ---- small Q-side prep (DVE) ----
              Qm = work.tile([128, LQ], F32R, tag="Qm", bufs=WORK_BUFS)
              nc.gpsimd.tensor_scalar(
                  out=Qm, in0=Qf, scalar1=w4mlu_sb, scalar2=None, op0=ALU.mult)
              Qaug = work.tile([128, LQ], F32R, tag="Qaug", bufs=WORK_BUFS)
              nc.gpsimd.tensor_scalar(
                  out=Qaug, in0=Qf, scalar1=w4mlu_sb, scalar2=w4C_sb,
                  op0=ALU.mult, op1=ALU.add)

              # ---- E2[c,q] = exp(C.T @ Qaug): c-tile j at cols 256j ----
              # first PE work of the batch: its front-ring slots were freed
              # by batch b-1's E1 psums (early), never by late-batch work
              E2 = work.tile([128, 16 * LQ], F32R, tag="E2", bufs=WORK_BUFS)
              for g in range(4):
                  ps = psum.tile([128, 1024], F32, tag="front", bufs=FRONT_BUFS)
                  for j in range(4):
                      ctile = g * 4 + j
                      nc.tensor.matmul(
                          ps[:, 256 * j:256 * (j + 1)],
                          Cr[:, 128 * ctile:128 * (ctile + 1)], Qaug,
                          start=True, stop=True)
                  nc.scalar.activation(
                      out=E2[:, 1024 * g:1024 * (g + 1)], in_=ps, func=AF.Exp)

              # ---- Ct = C.T (16 transposes), Pool copies ----
              Ct = work.tile([128, LC], F32R, tag="Ct", bufs=WORK_BUFS)
              for g in range(2):
                  ps_ct = psum.tile([128, 1024], F32, tag="front",
                                    bufs=FRONT_BUFS)
                  for j in range(8):
                      cj = g * 8 + j
                      nc.tensor.transpose(
                          ps_ct[:, 128 * j:128 * (j + 1)],
                          Cf[:, 128 * cj:128 * (cj + 1)], ident)
                  nc.scalar.copy(
                      out=Ct[:, 1024 * g:1024 * (g + 1)], in_=ps_ct)

              # ---- s (col-sums of E2, replicated) -> rs = 1/s row ----
              # s sits replicated in every partition, so 1/s works as a
              # free-axis row vector; no transposes, no DVE in the T-chain
              # beyond this early reciprocal
              ps_s = psum.tile([128, 512], F32, tag="small", bufs=SMALL_BUFS)
              for j in range(16):
                  nc.tensor.matmul(
                      ps_s[:, 0:256], ones, E2[:, 256 * j:256 * (j + 1)],
                      start=(j == 0), stop=(j == 15))
              rs = work.tile([128, LQ], F32, tag="rs", bufs=WORK_BUFS)
              nc.vector.reciprocal(out=rs, in_=ps_s[:, 0:256])

              # ---- MT^T = Ct.T @ E2; T^T = MT^T * (1/s) fused on Pool ----
              MTts = work.tile([128, LQ], F32R, tag="MTts", bufs=WORK_BUFS)
              ps_mt = psum.tile([128, 512], F32, tag="small", bufs=SMALL_BUFS)
              for j in range(16):
                  nc.tensor.matmul(
                      ps_mt[:, 0:256],
                      Ct[:, 128 * j:128 * (j + 1)], E2[:, 256 * j:256 * (j + 1)],
                      start=(j == 0), stop=(j == 15))
              nc.vector.scalar_tensor_tensor(
                  out=MTts, in0=ps_mt[:, 0:256], scalar=1.0, in1=rs,
                  op0=ALU.mult, op1=ALU.mult)

              # ---- T_sb[q,d] = transpose(T^T), Act copy from psum ----
              T_sb = work.tile([128, LQ], F32R, tag="T_sb", bufs=WORK_BUFS)
              ps_t = psum.tile([128, 512], F32, tag="small", bufs=SMALL_BUFS)
              for j in range(2):
                  nc.tensor.transpose(
                      ps_t[:, 128 * j:128 * (j + 1)],
                      MTts[:, 128 * j:128 * (j + 1)].bitcast(F32), ident)
              nc.scalar.copy(out=T_sb, in_=ps_t[:, 0:256])

              # ---- b[q] = Q.T @ w4Q (2 tiny matmuls), Act copy to SBUF ----
              ps_b = psum.tile([128, 512], F32, tag="back", bufs=BACK_BUFS)
              for qt in range(2):
                  nc.tensor.matmul(
                      ps_b[:, qt:qt + 1], Qf[:, 128 * qt:128 * (qt + 1)],
                      w4Q_sb, start=True, stop=True)
              b_sb = work.tile([128, 2], F32, tag="b_sb", bufs=WORK_BUFS)
              nc.scalar.copy(out=b_sb, in_=ps_b[:, 0:2])

              # ---- Qt = Q.T (2 transposes), Pool copy ----
              ps_qt = psum.tile([128, 512], F32, tag="back", bufs=BACK_BUFS)
              for j in range(2):
                  nc.tensor.transpose(
                      ps_qt[:, 128 * j:128 * (j + 1)],
                      Qf[:, 128 * j:128 * (j + 1)], ident)
              Qt = work.tile([128, LQ], F32R, tag="Qt", bufs=WORK_BUFS)
              nc.scalar.copy(out=Qt, in_=ps_qt[:, 0:256])

              # ---- E1^T[q,c] = exp(Qm.T @ C + b[q]): q-tile qt at 2048*qt ----
              E1 = work.tile([128, 2 * LC], F32R, tag="E1", bufs=WORK_BUFS)
              for qt in range(2):
                  for g in range(2):
                      ps = psum.tile([128, 1024], F32, tag="front",
                                     bufs=FRONT_BUFS)
                      for cc in range(2):
                          c0 = 1024 * g + 512 * cc
                          nc.tensor.matmul(
                              ps[:, 512 * cc:512 * (cc + 1)],
                              Qm[:, 128 * qt:128 * (qt + 1)],
                              Cr[:, c0:c0 + 512],
                              start=True, stop=True)
                      nc.scalar.activation(
                          out=E1[:, 2048 * qt + 1024 * g:2048 * qt + 1024 * (g + 1)],
                          in_=ps, func=AF.Exp, bias=b_sb[:, qt:qt + 1])

              # ---- r (row-sums of E1 over q, replicated) -> rbi = 1/r ----
              rbi = work.tile([128, LC], F32, tag="rbi", bufs=WORK_BUFS)
              for gc in range(4):
                  c0 = 512 * gc
                  ps = psum.tile([128, 512], F32, tag="back", bufs=BACK_BUFS)
                  for qt in range(2):
                      nc.tensor.matmul(
                          ps, ones, E1[:, 2048 * qt + c0:2048 * qt + c0 + 512],
                          start=(qt == 0), stop=(qt == 1))
                  nc.vector.reciprocal_approx_fast(
                      out=rbi[:, c0:c0 + 512], in_=ps)

              # Crbi = C * (1/r) on Pool, split per half for earlier starts
              Crbi = work.tile([128, LC], F32, tag="Crbi", bufs=WORK_BUFS)
              for g in range(2):
                  sl = slice(1024 * g, 1024 * (g + 1))
                  nc.gpsimd.tensor_mul(out=Crbi[:, sl], in0=Cf[:, sl],
                                       in1=rbi[:, sl])

              # ---- per column-half: MA -> out1/out2, MB -> out3 ----
              # per-plane staging tiles + per-plane stores: each plane ships
              # as soon as its two muls finish (keeps DMA fed, short tail)
              for g in range(2):
                with G(6 + g):
                  sl = slice(1024 * g, 1024 * (g + 1))
                  P1 = io.tile([128, 1024], F32, tag="outp", bufs=OUT_BUFS)
                  P2 = io.tile([128, 1024], F32, tag="outp", bufs=OUT_BUFS)
                  P3 = io.tile([128, 1024], F32, tag="outp", bufs=OUT_BUFS)
                  for cc in range(2):
                      c0 = 1024 * g + 512 * cc
                      scc = slice(512 * cc, 512 * (cc + 1))
                      psA = psum.tile([128, 512], F32, tag="back",
                                      bufs=BACK_BUFS)
                      for qt in range(2):
                          nc.tensor.matmul(
                              psA,
                              Qt[:, 128 * qt:128 * (qt + 1)],
                              E1[:, 2048 * qt + c0:2048 * qt + c0 + 512],
                              start=(qt == 0), stop=(qt == 1))
                      nc.vector.tensor_mul(out=P1[:, scc], in0=psA,
                                           in1=rbi[:, c0:c0 + 512])
                      nc.vector.tensor_mul(out=P2[:, scc], in0=psA,
                                           in1=Crbi[:, c0:c0 + 512])
                      nc.sync.dma_start(out=out_ext[b, 128:256, c0:c0 + 512],
                                        in_=P1[:, scc])
                      nc.sync.dma_start(out=out_ext[b, 256:384, c0:c0 + 512],
                                        in_=P2[:, scc])
                  for cc in range(2):
                      c0 = 1024 * g + 512 * cc
                      scc = slice(512 * cc, 512 * (cc + 1))
                      psB = psum.tile([128, 512], F32, tag="back",
                                      bufs=BACK_BUFS)
                      for qt in range(2):
                          nc.tensor.matmul(
                              psB,
                              T_sb[:, 128 * qt:128 * (qt + 1)],
                              E1[:, 2048 * qt + c0:2048 * qt + c0 + 512],
                              start=(qt == 0), stop=(qt == 1))
                      nc.vector.tensor_mul(out=P3[:, scc], in0=psB,
                                           in1=Crbi[:, c0:c0 + 512])
                      nc.sync.dma_start(out=out_ext[b, 384:512, c0:c0 + 512],
                                        in_=P3[:, scc])

    nc.compile()
    return nc


_NC = {}


def _get_nc(reps=1, hw_loop=False):
    key = (reps, hw_loop)
    if key not in _NC:
        _NC[key] = build_nc(reps, hw_loop)
    return _NC[key]


def make_in_maps(C, Q, w4C, w4Q, w4mlu):
    C = np.ascontiguousarray(np.asarray(C), dtype=np.float32)
    Q = np.ascontiguousarray(np.asarray(Q), dtype=np.float32)
    w4C = np.ascontiguousarray(np.asarray(w4C), dtype=np.float32).reshape(D, 1)
    w4Q = np.ascontiguousarray(np.asarray(w4Q), dtype=np.float32).reshape(D, 1)
    w4mlu = np.ascontiguousarray(np.asarray(w4mlu), dtype=np.float32).reshape(D, 1)
    in_maps = []
    for i in range(NCORES):
        sl = slice(i * BPC, (i + 1) * BPC)
        in_maps.append({
            "C": np.ascontiguousarray(C[sl]),
            "Q": np.ascontiguousarray(Q[sl]),
            "w4C": w4C, "w4Q": w4Q, "w4mlu": w4mlu,
        })
    return in_maps


def run(C, Q, w4C, w4Q, w4mlu, trace=False, tmpdir=None):
    from concourse.bass_utils import run_bass_kernel_spmd
    nc = _get_nc()
    in_maps = make_in_maps(C, Q, w4C, w4Q, w4mlu)
    res = run_bass_kernel_spmd(
        nc, in_maps, list(range(NCORES)), trace=trace, tmpdir=tmpdir)
    out = np.concatenate(
        [res.results[i]["out"] for i in range(NCORES)], axis=0)
    return out, res


def kernel(C, Q, Cmask=None, Qmask=None, w4C=None, w4Q=None, w4mlu=None,
           bias=None, **_unused):
    # Cmask/Qmask are all-ones in this problem and bias cancels exactly in
    # every output (softmax shift invariance), so neither reaches the device.
    out, _ = run(C, Q, w4C, w4Q, w4mlu)
    return out
